# revision 39
# baseline (speedup 1.0000x reference)
"""AttentionBlock (GroupNorm + 8-head self-attention + proj + residual) on 8 trn2 cores.

Sharding: data-parallel over batch (B=8 -> 1 sample per core). No collectives.

Per-core layout (one sample, C=512, N=H*W=1024):
  x [C, N] channels-on-partitions, 4 c-tiles of [128, 1024].
  GroupNorm: per-channel mean/var via bn_stats, group-reduce via tiny matmul with
    a 0/1 group-indicator, rsqrt via Sqrt+reciprocal, broadcast back via tiny
    matmul, fused apply via tensor_scalar.
  qkv: bf16 matmuls against pre-transposed weights (fp32 accumulate in PSUM).
    q,k stay [C, N]; v is produced directly transposed (vT [N, C]) by swapping
    matmul operands, with a ones-column per head appended.
  Attention per head h (hd=64): scores are computed TRANSPOSED,
    ST[m, n] = k_h^T q_h (K=64 contraction), so softmax's reduction direction
    (over m) lands on the partition axis, which the AV matmul contracts anyway:
    the ones-column in vT gives the softmax denominator for free as row 64 of
    the AV output. exp on ACT with fused 1/8 scale reads scores straight from
    PSUM (writing bf16). Normalize = fast-reciprocal + gpsimd
    partition-broadcast + one elementwise multiply.
  proj: bf16 matmul + per-partition bias + fp32 residual add, DMA out.

  Scheduling: one flat software-pipelined stream. Scores+exp run LA steps
  ahead of the trailing head-A AV; head-B AV blasts through retained exp
  tiles after head A normalizes (only one PSUM AV accumulator live, so the
  scores pool gets 3 buffers and ACT -- the 73us exp floor and true
  bottleneck -- never starves). Next pair's q/k are produced in 2-matmul
  chunks as PE filler; proj k-steps 0..2 pre-accumulate during the last
  normalize; a few zero matmuls keep the PE clock-gate warm through the
  DMA-bound startup.
"""

import sys

sys.path.insert(0, "/opt/trn_rl_repo")

import contextlib

import ml_dtypes
import numpy as np

import concourse.bass as bass
import concourse.tile as tile
from concourse import bacc, mybir
from concourse.bass_utils import run_bass_kernel_spmd

f32 = mybir.dt.float32
f32r = mybir.dt.float32r
bf16 = mybir.dt.bfloat16
AF = mybir.ActivationFunctionType
OP = mybir.AluOpType

C = 512
N = 1024
NHEADS = 8
HD = 64
GROUPS = 32
GSIZE = 16  # channels per group
CT = 4  # c-tiles of 128
MT = 8  # m(n)-tiles of 128
PAIRS = 4  # head pairs (2 heads = 128 channels per c-tile)
EPS = 1e-5
NCHUNK = 512  # matmul moving-dim chunk
P = 128


def build_program():
    nc = bacc.Bacc("TRN2", target_bir_lowering=False, debug=True)

    x_d = nc.dram_tensor("x", [C, N], f32, kind="ExternalInput")
    wqkvT_d = nc.dram_tensor("wqkvT", [C, 3 * C], bf16, kind="ExternalInput")
    wpT_d = nc.dram_tensor("wpT", [C, C], bf16, kind="ExternalInput")
    qkb_d = nc.dram_tensor("qkb", [P, 8], f32, kind="ExternalInput")
    vb_d = nc.dram_tensor("vb", [1, C], bf16, kind="ExternalInput")
    pb_d = nc.dram_tensor("pb", [P, CT], f32, kind="ExternalInput")
    gnw_d = nc.dram_tensor("gnw", [P, CT], f32, kind="ExternalInput")
    gnb_d = nc.dram_tensor("gnb", [P, CT], f32, kind="ExternalInput")
    gmap_d = nc.dram_tensor("gmap", [P, 8], f32, kind="ExternalInput")
    gmapT_d = nc.dram_tensor("gmapT", [8, P], f32, kind="ExternalInput")
    ones8_d = nc.dram_tensor("ones8", [P, 8], bf16, kind="ExternalInput")
    ones1_d = nc.dram_tensor("ones1", [1, P], bf16, kind="ExternalInput")
    out_d = nc.dram_tensor("out", [C, N], f32, kind="ExternalOutput")

    with tile.TileContext(nc) as tc, contextlib.ExitStack() as ctx:
        consts = ctx.enter_context(tc.tile_pool(name="consts", bufs=1))
        xp = ctx.enter_context(tc.tile_pool(name="xp", bufs=CT))
        xnp = ctx.enter_context(tc.tile_pool(name="xnp", bufs=CT))
        qkp = ctx.enter_context(tc.tile_pool(name="qkp", bufs=4))
        vtp = ctx.enter_context(tc.tile_pool(name="vtp", bufs=MT))
        wp = ctx.enter_context(tc.tile_pool(name="wp", bufs=CT))
        wpp = ctx.enter_context(tc.tile_pool(name="wpp", bufs=CT))
        attp = ctx.enter_context(tc.tile_pool(name="attp", bufs=CT))
        expp = ctx.enter_context(tc.tile_pool(name="expp", bufs=14))
        dvp = ctx.enter_context(tc.tile_pool(name="dvp", bufs=2))
        gnp = ctx.enter_context(tc.tile_pool(name="gnp", bufs=4))
        outp = ctx.enter_context(tc.tile_pool(name="outp", bufs=2))

        psum_big = ctx.enter_context(tc.tile_pool(name="psum_big", bufs=3, space="PSUM"))

        # ---- input DMAs: x first (GroupNorm starts on it), GN consts,
        # qkv weights on the second HWDGE ring, everything else after ----
        x_tiles = []
        for t in range(CT):
            xt = xp.tile([P, N], f32, tag="x")
            for hh in range(2):
                nc.sync.dma_start(
                    xt[:, hh * NCHUNK:(hh + 1) * NCHUNK],
                    x_d[t * P:(t + 1) * P, hh * NCHUNK:(hh + 1) * NCHUNK],
                )
            x_tiles.append(xt)

        w_tiles = []
        for t in range(CT):
            wt = wp.tile([P, 3 * C], bf16, tag="w")
            nc.sync.dma_start(wt[:], wqkvT_d[t * P:(t + 1) * P, :])
            w_tiles.append(wt)

        gnw_t = consts.tile([P, CT], f32)
        nc.sync.dma_start(gnw_t[:], gnw_d[:])
        gnb_t = consts.tile([P, CT], f32)
        nc.sync.dma_start(gnb_t[:], gnb_d[:])
        gmap_t = consts.tile([P, 8], f32)
        nc.sync.dma_start(gmap_t[:], gmap_d[:])
        gmapT_t = consts.tile([8, P], f32)
        nc.sync.dma_start(gmapT_t[:], gmapT_d[:])
        qkb_t = consts.tile([P, 8], f32)
        nc.sync.dma_start(qkb_t[:], qkb_d[:])
        vb_t = consts.tile([1, C], bf16)
        nc.sync.dma_start(vb_t[:], vb_d[:])
        pb_t = consts.tile([P, CT], f32)
        nc.sync.dma_start(pb_t[:], pb_d[:])
        ones8_t = consts.tile([P, 8], bf16)
        nc.sync.dma_start(ones8_t[:], ones8_d[:])
        ones1_t = consts.tile([1, P], bf16)
        nc.sync.dma_start(ones1_t[:], ones1_d[:])
        eps_t = consts.tile([8, 1], f32)
        nc.vector.memset(eps_t[:], EPS)
        # preload the sqrt ACT table set at t=0 (overlaps input DMAs)
        warm_t = consts.tile([1, 1], f32)
        nc.vector.memset(warm_t[:], 1.0)
        nc.scalar.activation(out=warm_t[:], in_=warm_t[:], func=AF.Sqrt)
        # zeros tile for PE-warming matmuls (keeps the HAM clock-gate at full
        # rate through the DMA/GroupNorm-gated startup window)
        zd_t = consts.tile([P, NCHUNK], f32)
        nc.vector.memset(zd_t[:], 0.0)

        wp_tiles = []
        for t in range(CT):
            wt = wpp.tile([P, C], bf16, tag="wp")
            nc.scalar.dma_start(wt[:], wpT_d[t * P:(t + 1) * P, :])
            wp_tiles.append(wt)

        def pe_warm(n):
            for _ in range(n):
                dp = psum_big.tile([P, N], f32, tag="big", name="dummy")
                nc.tensor.matmul(
                    dp[:, 0:NCHUNK], zd_t[:, 0:P], zd_t[:], start=True, stop=True
                )

        def pe_tick():
            # tiny activity pulse: keeps the HAM idle-window from firing
            dp = psum_big.tile([P, N], f32, tag="big", name="dummy")
            nc.tensor.matmul(
                dp[:, 0:64], zd_t[:, 0:P], zd_t[:, 0:64], start=True, stop=True
            )

        # ---- GroupNorm ----
        with tc.tile_pool(name="psum_tiny", bufs=1, space="PSUM") as psum_tiny:
            pe_warm(3)
            xn_tiles = []
            for t in range(CT):
                xt = x_tiles[t]
                xv = xt[:].rearrange("p (s f) -> p s f", s=2)
                st = gnp.tile([P, 2, 6], f32, tag="bnst")
                nc.vector.bn_stats(out=st[:, 0, :], in_=xv[:, 0, :])
                nc.vector.bn_stats(out=st[:, 1, :], in_=xv[:, 1, :])
                mv = gnp.tile([P, 2], f32, tag="bnmv")
                nc.vector.bn_aggr(out=mv[:], in_=st[:])
                # cst = [mean_c, var_c + mean_c^2] = [E[x], E[x^2]] per channel
                cst = gnp.tile([P, 2], f32, tag="cst")
                nc.vector.tensor_copy(cst[:, 0:1], mv[:, 0:1])
                nc.vector.scalar_tensor_tensor(
                    out=cst[:, 1:2], in0=mv[:, 0:1], scalar=mv[:, 0:1],
                    in1=mv[:, 1:2], op0=OP.mult, op1=OP.add,
                )
                # group sums (over the 16 channels of each of this tile's 8 groups)
                pgs = psum_tiny.tile([8, 2], f32, tag="pgs")
                nc.tensor.matmul(pgs[:], gmap_t[:], cst[:], start=True, stop=True)
                # mu_g, rstd_g on 8 partitions
                gr = gnp.tile([8, 2], f32, tag="gr")
                nc.vector.tensor_scalar_mul(gr[:, 0:2], pgs[:, 0:2], 1.0 / GSIZE)
                musq = gnp.tile([8, 1], f32, tag="musq")
                nc.vector.tensor_mul(musq[:], gr[:, 0:1], gr[:, 0:1])
                var = gnp.tile([8, 1], f32, tag="var")
                nc.vector.tensor_sub(var[:], gr[:, 1:2], musq[:])
                std = gnp.tile([8, 1], f32, tag="std")
                nc.scalar.activation(out=std[:], in_=var[:], func=AF.Sqrt, bias=eps_t[:])
                nc.vector.reciprocal(gr[:, 1:2], std[:])
                # broadcast mu/rstd back to the tile's 128 channels
                pbc = psum_tiny.tile([P, 2], f32, tag="pbc")
                nc.tensor.matmul(pbc[:], gmapT_t[:], gr[:], start=True, stop=True)
                scale_c = gnp.tile([P, 1], f32, tag="scale_c")
                nc.vector.tensor_mul(scale_c[:], pbc[:, 1:2], gnw_t[:, t:t + 1])
                mss = gnp.tile([P, 1], f32, tag="mss")
                nc.vector.tensor_mul(mss[:], pbc[:, 0:1], scale_c[:])
                bias_c = gnp.tile([P, 1], f32, tag="bias_c")
                nc.vector.tensor_sub(bias_c[:], gnb_t[:, t:t + 1], mss[:])
                xnt = xnp.tile([P, N], bf16, tag="xn")
                nc.vector.tensor_scalar(
                    out=xnt[:], in0=xt[:], scalar1=scale_c[:], scalar2=bias_c[:],
                    op0=OP.mult, op1=OP.add,
                )
                xn_tiles.append(xnt)
                pe_warm(2)

        with tc.tile_pool(name="psum_av", bufs=1, space="PSUM") as psum_av:

            # ---- qkv helpers ----
            def emit_vt_tile(i):
                """vT tile [128, 8*65]; for head h cols 65h..65h+64 hold v
                channels 64h..64h+64, col 65h+64 holds ones (denominator)."""
                ps = psum_big.tile([P, N], f32, tag="big", name=f"vtps{i}")
                pv = ps[:, 0:NCHUNK]
                for kk in range(CT):
                    nc.tensor.matmul(
                        pv,
                        xn_tiles[kk][:, i * P:(i + 1) * P],
                        w_tiles[kk][:, 2 * C:3 * C],
                        start=(kk == 0), stop=False,
                    )
                nc.tensor.matmul(pv, ones1_t[:], vb_t[:], start=False, stop=True)
                vt = vtp.tile([P, NHEADS, HD + 1], bf16, tag="vt", name=f"vt{i}")
                nc.vector.tensor_copy(
                    vt[:, :, 0:HD], pv.rearrange("p (h d) -> p h d", h=NHEADS)
                )
                nc.vector.tensor_copy(vt[:, :, HD:HD + 1], ones8_t[:, :, None])
                return vt

            att_tiles = []

            def emit_scores(p, i, q_t, k_t):
                """transposed scores for heads (2p, 2p+1), m-tile i -> PSUM pair."""
                pss = []
                for h in range(2):
                    ps = psum_big.tile([P, N], f32, tag="big")
                    lo = h * HD
                    for j in range(2):
                        nc.tensor.matmul(
                            ps[:, j * NCHUNK:(j + 1) * NCHUNK],
                            k_t[lo:lo + HD, i * P:(i + 1) * P],
                            q_t[lo:lo + HD, j * NCHUNK:(j + 1) * NCHUNK],
                            start=True, stop=True,
                        )
                    pss.append(ps)
                return pss

            def emit_exp(ps_pair):
                es = []
                for ps in ps_pair:
                    e = expp.tile([P, N], bf16, tag="exp")
                    nc.scalar.activation(out=e[:], in_=ps[:], func=AF.Exp, scale=1.0 / 8.0)
                    es.append(e)
                return es

            # ---- flat software-pipelined attention stream ----
            # Per A-phase step: AV_A (trailing the exp stream), one scores+exp
            # step LA ahead, and small PE filler chunks (vT production during
            # pair 0, next pair's q/k in 4-matmul chunks during steps 0..3) so
            # the PE never executes a large blocking block and the big-PSUM
            # pool keeps feeding ACT. Head B's 16 matmuls blast through the
            # retained exp tiles after head A normalizes.
            LA = 3
            steps = [(p, i) for p in range(PAIRS) for i in range(MT)]
            exps = {}
            emitted = 0

            qk_state = {}  # p -> dict(ps=[q_ps,k_ps], sb=[q_sb,k_sb], chunk=int)

            def qk_begin(p):
                qk_state[p] = {"chunk": 0, "ps": None, "sb": []}

            def qk_chunk(p):
                """Emit 2 of the 16 qk matmuls for pair p; q fully first, then
                k, so only one big-PSUM slot is held at a time."""
                st = qk_state[p]
                c = st["chunk"]
                if c >= 8:
                    return
                st["chunk"] = c + 1
                which, cc = c // 4, c % 4
                off = which * C + p * P
                if cc == 0:
                    st["ps"] = psum_big.tile(
                        [P, N], f32, tag="big", name=f"qkps{p}_{which}"
                    )
                ps = st["ps"]
                j, kks = cc // 2, (cc % 2) * 2
                for kk in (kks, kks + 1):
                    nc.tensor.matmul(
                        ps[:, j * NCHUNK:(j + 1) * NCHUNK],
                        w_tiles[kk][:, off:off + P],
                        xn_tiles[kk][:, j * NCHUNK:(j + 1) * NCHUNK],
                        start=(kk == 0), stop=(kk == CT - 1),
                    )
                if cc == 3:
                    sb = qkp.tile([P, N], bf16, tag="qk", name=f"qk{p}_{which}")
                    nc.vector.tensor_scalar_add(
                        sb[:], ps[:], qkb_t[:, which * 4 + p:which * 4 + p + 1]
                    )
                    st["sb"].append(sb)

            def qk_force(p):
                while qk_state[p]["chunk"] < 8:
                    qk_chunk(p)

            def ensure_scores(n):
                nonlocal emitted
                while emitted < min(n, len(steps)):
                    p2, i2 = steps[emitted]
                    qk_force(p2)
                    exps[(p2, i2)] = emit_exp(
                        emit_scores(p2, i2, *qk_state[p2]["sb"])
                    )
                    emitted += 1

            def emit_av(avt, p, i, h, start, stop):
                e = exps.pop((p, i))[h] if h == 1 else exps[(p, i)][h]
                for j in range(2):
                    nc.tensor.matmul(
                        avt[:, j * NCHUNK:(j + 1) * NCHUNK],
                        vt_tiles[i][:, 2 * p + h, :],
                        e[:, j * NCHUNK:(j + 1) * NCHUNK],
                        start=start, stop=stop,
                    )

            def emit_norm(att, avt, h):
                dinv = dvp.tile([1, N], f32, tag="dinv", name=f"dinv{h}")
                nc.vector.tensor_copy(dinv[:], avt[HD:HD + 1, :])
                nc.vector.reciprocal_approx_fast(dinv[:], dinv[:])
                dinvb = dvp.tile([HD, N], f32, tag="dinvb", name=f"dinvb{h}")
                nc.gpsimd.partition_broadcast(dinvb[:], dinv[:])
                nc.vector.tensor_mul(
                    att[h * HD:(h + 1) * HD, :], avt[0:HD, :], dinvb[:]
                )

            vt_tiles = [None] * MT
            proj_ps = {}
            qk_begin(0)
            for _ in range(8):
                qk_chunk(0)
                pe_tick()
            ensure_scores(LA)
            for i in range(MT):
                vt_tiles[i] = emit_vt_tile(i)
                pe_tick()
                # keep ACT's score queue topped up through the vt block
                if i in (2, 4, 6):
                    ensure_scores(LA + 1 + i // 2)
            for p in range(PAIRS):
                att = attp.tile([P, N], bf16, tag="att", name=f"att{p}")
                # head A trails the exp stream
                avt = psum_av.tile([HD + 1, N], f32, tag="av", name=f"avA{p}")
                for i in range(MT):
                    ensure_scores(p * MT + i + 1 + LA)
                    if p + 1 < PAIRS:
                        if i == 0:
                            qk_begin(p + 1)
                        qk_chunk(p + 1)
                        if i >= 2:
                            qk_chunk(p + 1)
                    emit_av(avt, p, i, 0, start=(i == 0), stop=(i == MT - 1))
                emit_norm(att, avt, 0)
                if p == PAIRS - 1:
                    # pre-accumulate proj k-steps 0..2 for o-tiles 0..2 -- keeps
                    # the PE busy while the last normalize chains run on DVE
                    for o in range(3):
                        pp = psum_big.tile([P, N], f32, tag="big", name=f"projps{o}")
                        for kk in range(CT - 1):
                            for j in range(2):
                                nc.tensor.matmul(
                                    pp[:, j * NCHUNK:(j + 1) * NCHUNK],
                                    wp_tiles[kk][:, o * P:(o + 1) * P],
                                    att_tiles[kk][:, j * NCHUNK:(j + 1) * NCHUNK],
                                    start=(kk == 0), stop=False,
                                )
                        proj_ps[o] = pp
                # head B blasts through the retained exp tiles
                avt = psum_av.tile([HD + 1, N], f32, tag="av", name=f"avB{p}")
                for i in range(MT):
                    emit_av(avt, p, i, 1, start=(i == 0), stop=(i == MT - 1))
                    if i % 3 == 2:
                        ensure_scores(p * MT + MT + i // 3 + 1 + LA)
                emit_norm(att, avt, 1)
                att_tiles.append(att)

            # ---- proj + residual ----
            for t in range(CT):
                if t in proj_ps:
                    ps = proj_ps[t]
                else:
                    ps = psum_big.tile([P, N], f32, tag="big", name=f"projfull{t}")
                    for kk in range(CT - 1):
                        for j in range(2):
                            nc.tensor.matmul(
                                ps[:, j * NCHUNK:(j + 1) * NCHUNK],
                                wp_tiles[kk][:, t * P:(t + 1) * P],
                                att_tiles[kk][:, j * NCHUNK:(j + 1) * NCHUNK],
                                start=(kk == 0), stop=False,
                            )
                for j in range(2):
                    nc.tensor.matmul(
                        ps[:, j * NCHUNK:(j + 1) * NCHUNK],
                        wp_tiles[CT - 1][:, t * P:(t + 1) * P],
                        att_tiles[CT - 1][:, j * NCHUNK:(j + 1) * NCHUNK],
                        start=False, stop=True,
                    )
                ot = outp.tile([P, N], f32, tag="ot")
                nc.vector.scalar_tensor_tensor(
                    out=ot[:], in0=ps[:], scalar=pb_t[:, t:t + 1],
                    in1=x_tiles[t][:], op0=OP.add, op1=OP.add,
                )
                nc.sync.dma_start(out_d[t * P:(t + 1) * P, :], ot[:])

    nc.compile()
    return nc


_CACHE = {}


def _get_program():
    if "nc" not in _CACHE:
        _CACHE["nc"] = build_program()
    return _CACHE["nc"]


def make_in_maps(x, gn_w, gn_b, qkv_w, qkv_b, proj_w, proj_b):
    B = x.shape[0]
    f = np.float32
    wqkvT = np.ascontiguousarray(qkv_w.T).astype(ml_dtypes.bfloat16)  # [512, 1536]
    wpT = np.ascontiguousarray(proj_w.T).astype(ml_dtypes.bfloat16)  # [512, 512]
    qkb = np.ascontiguousarray(np.asarray(qkv_b[:2 * C], f).reshape(8, P).T)  # [128, 8]
    vb = np.asarray(qkv_b[2 * C:], np.float32).reshape(1, C).astype(ml_dtypes.bfloat16)
    pb = np.ascontiguousarray(np.asarray(proj_b, f).reshape(CT, P).T)  # [128, 4]
    gnw = np.ascontiguousarray(np.asarray(gn_w, f).reshape(CT, P).T)
    gnb = np.ascontiguousarray(np.asarray(gn_b, f).reshape(CT, P).T)
    # group indicator: gmap[p, j] = 1 if channel p belongs to (tile-local) group j
    gmap = np.zeros((P, 8), f)
    gmap[np.arange(P), np.arange(P) // GSIZE] = 1.0
    gmapT = np.ascontiguousarray(gmap.T)
    ones8 = np.ones((P, 8), ml_dtypes.bfloat16)
    ones1 = np.ones((1, P), ml_dtypes.bfloat16)
    shared = dict(
        wqkvT=wqkvT, wpT=wpT, qkb=qkb, vb=vb, pb=pb, gnw=gnw, gnb=gnb,
        gmap=gmap, gmapT=gmapT, ones8=ones8, ones1=ones1,
    )
    xs = np.asarray(x, f).reshape(B, C, N)
    return [dict(shared, x=np.ascontiguousarray(xs[i])) for i in range(B)]


def run(in_maps, trace=False, **kw):
    nc = _get_program()
    return run_bass_kernel_spmd(nc, in_maps, core_ids=list(range(len(in_maps))), trace=trace, **kw)


def kernel(x, gn_w, gn_b, qkv_w, qkv_b, proj_w, proj_b):
    x = np.asarray(x)
    B, c, h, w = x.shape
    in_maps = make_in_maps(x, gn_w, gn_b, qkv_w, qkv_b, proj_w, proj_b)
    res = run(in_maps)
    out = np.stack([res.results[i]["out"].reshape(c, h, w) for i in range(B)])
    return out.astype(np.float32)


# revision 40
# speedup vs baseline: 1.2690x; 1.2690x over previous
"""AttentionBlock (GroupNorm + 8-head self-attention + proj + residual) on 8 trn2 cores.

Sharding: data-parallel over batch (B=8 -> 1 sample per core). No collectives.

Per-core layout (one sample, C=512, N=H*W=1024):
  x [C, N] channels-on-partitions, 4 c-tiles of [128, 1024].
  GroupNorm: per-channel mean/var via bn_stats, group-reduce via tiny matmul with
    a 0/1 group-indicator, rsqrt via Sqrt+reciprocal, broadcast back via tiny
    matmul, fused apply via tensor_scalar.
  qkv: bf16 matmuls against pre-transposed weights (fp32 accumulate in PSUM).
    q,k stay [C, N]; v is produced directly transposed (vT [N, C]) by swapping
    matmul operands, with a ones-column per head appended.
  Attention per head h (hd=64): scores are computed TRANSPOSED,
    ST[m, n] = k_h^T q_h (K=64 contraction), so softmax's reduction direction
    (over m) lands on the partition axis, which the AV matmul contracts anyway:
    the ones-column in vT gives the softmax denominator for free as row 64 of
    the AV output. exp on ACT with fused 1/8 scale reads scores straight from
    PSUM (writing bf16). Normalize = fast-reciprocal + gpsimd
    partition-broadcast + one elementwise multiply.
  proj: bf16 matmul + per-partition bias + fp32 residual add, DMA out.

  Scheduling: one flat software-pipelined stream. Scores+exp run LA steps
  ahead of the trailing head-A AV; head-B AV blasts through retained exp
  tiles after head A normalizes (only one PSUM AV accumulator live, so the
  scores pool gets 3 buffers and ACT -- the 73us exp floor and true
  bottleneck -- never starves). Next pair's q/k are produced in 2-matmul
  chunks as PE filler; proj k-steps 0..2 pre-accumulate during the last
  normalize; a few zero matmuls keep the PE clock-gate warm through the
  DMA-bound startup.
"""

import sys

sys.path.insert(0, "/opt/trn_rl_repo")

import contextlib

import ml_dtypes
import numpy as np

import concourse.bass as bass
import concourse.tile as tile
from concourse import bacc, mybir
from concourse.bass_utils import run_bass_kernel_spmd

f32 = mybir.dt.float32
f32r = mybir.dt.float32r
bf16 = mybir.dt.bfloat16
AF = mybir.ActivationFunctionType
OP = mybir.AluOpType

C = 512
N = 1024
NHEADS = 8
HD = 64
GROUPS = 32
GSIZE = 16  # channels per group
CT = 4  # c-tiles of 128
MT = 8  # m(n)-tiles of 128
PAIRS = 4  # head pairs (2 heads = 128 channels per c-tile)
EPS = 1e-5
NCHUNK = 512  # matmul moving-dim chunk
P = 128


def build_program():
    nc = bacc.Bacc("TRN2", target_bir_lowering=False, debug=True)

    x_d = nc.dram_tensor("x", [C, N], f32, kind="ExternalInput")
    wqkvT_d = nc.dram_tensor("wqkvT", [C, 3 * C], bf16, kind="ExternalInput")
    wpT_d = nc.dram_tensor("wpT", [C, C], bf16, kind="ExternalInput")
    qkb_d = nc.dram_tensor("qkb", [P, 8], f32, kind="ExternalInput")
    vb_d = nc.dram_tensor("vb", [1, C], bf16, kind="ExternalInput")
    pb_d = nc.dram_tensor("pb", [P, CT], f32, kind="ExternalInput")
    gnw_d = nc.dram_tensor("gnw", [P, CT], f32, kind="ExternalInput")
    gnb_d = nc.dram_tensor("gnb", [P, CT], f32, kind="ExternalInput")
    gmap_d = nc.dram_tensor("gmap", [P, 8], f32, kind="ExternalInput")
    gmapT_d = nc.dram_tensor("gmapT", [8, P], f32, kind="ExternalInput")
    ones8_d = nc.dram_tensor("ones8", [P, 8], bf16, kind="ExternalInput")
    ones1_d = nc.dram_tensor("ones1", [1, P], bf16, kind="ExternalInput")
    out_d = nc.dram_tensor("out", [C, N], f32, kind="ExternalOutput")

    with tile.TileContext(nc) as tc, contextlib.ExitStack() as ctx:
        consts = ctx.enter_context(tc.tile_pool(name="consts", bufs=1))
        xp = ctx.enter_context(tc.tile_pool(name="xp", bufs=CT))
        xnp = ctx.enter_context(tc.tile_pool(name="xnp", bufs=CT))
        qkp = ctx.enter_context(tc.tile_pool(name="qkp", bufs=4))
        vtp = ctx.enter_context(tc.tile_pool(name="vtp", bufs=MT))
        wp = ctx.enter_context(tc.tile_pool(name="wp", bufs=CT))
        wpp = ctx.enter_context(tc.tile_pool(name="wpp", bufs=CT))
        attp = ctx.enter_context(tc.tile_pool(name="attp", bufs=CT))
        expp = ctx.enter_context(tc.tile_pool(name="expp", bufs=14))
        dvp = ctx.enter_context(tc.tile_pool(name="dvp", bufs=2))
        gnp = ctx.enter_context(tc.tile_pool(name="gnp", bufs=4))
        outp = ctx.enter_context(tc.tile_pool(name="outp", bufs=2))

        psum_big = ctx.enter_context(tc.tile_pool(name="psum_big", bufs=3, space="PSUM"))

        # ---- input DMAs: x first (GroupNorm starts on it), GN consts,
        # qkv weights on the second HWDGE ring, everything else after ----
        x_tiles = []
        for t in range(CT):
            xt = xp.tile([P, N], f32, tag="x")
            for hh in range(2):
                nc.sync.dma_start(
                    xt[:, hh * NCHUNK:(hh + 1) * NCHUNK],
                    x_d[t * P:(t + 1) * P, hh * NCHUNK:(hh + 1) * NCHUNK],
                )
            x_tiles.append(xt)

        w_tiles = []
        for t in range(CT):
            wt = wp.tile([P, 3 * C], bf16, tag="w")
            nc.sync.dma_start(wt[:], wqkvT_d[t * P:(t + 1) * P, :])
            w_tiles.append(wt)

        gnw_t = consts.tile([P, CT], f32)
        nc.sync.dma_start(gnw_t[:], gnw_d[:])
        gnb_t = consts.tile([P, CT], f32)
        nc.sync.dma_start(gnb_t[:], gnb_d[:])
        gmap_t = consts.tile([P, 8], f32)
        nc.sync.dma_start(gmap_t[:], gmap_d[:])
        gmapT_t = consts.tile([8, P], f32)
        nc.sync.dma_start(gmapT_t[:], gmapT_d[:])
        qkb_t = consts.tile([P, 8], f32)
        nc.sync.dma_start(qkb_t[:], qkb_d[:])
        vb_t = consts.tile([1, C], bf16)
        nc.sync.dma_start(vb_t[:], vb_d[:])
        pb_t = consts.tile([P, CT], f32)
        nc.sync.dma_start(pb_t[:], pb_d[:])
        ones8_t = consts.tile([P, 8], bf16)
        nc.sync.dma_start(ones8_t[:], ones8_d[:])
        ones1_t = consts.tile([1, P], bf16)
        nc.sync.dma_start(ones1_t[:], ones1_d[:])
        eps_t = consts.tile([8, 1], f32)
        nc.vector.memset(eps_t[:], EPS)
        # preload the sqrt ACT table set at t=0 (overlaps input DMAs)
        warm_t = consts.tile([1, 1], f32)
        nc.vector.memset(warm_t[:], 1.0)
        nc.scalar.activation(out=warm_t[:], in_=warm_t[:], func=AF.Sqrt)
        # zeros tile for PE-warming matmuls (keeps the HAM clock-gate at full
        # rate through the DMA/GroupNorm-gated startup window)
        zd_t = consts.tile([P, NCHUNK], f32)
        nc.vector.memset(zd_t[:], 0.0)

        wp_tiles = []
        for t in range(CT):
            wt = wpp.tile([P, C], bf16, tag="wp")
            nc.scalar.dma_start(wt[:], wpT_d[t * P:(t + 1) * P, :])
            wp_tiles.append(wt)

        def pe_warm(n):
            for _ in range(n):
                dp = psum_big.tile([P, N], f32, tag="big", name="dummy")
                nc.tensor.matmul(
                    dp[:, 0:NCHUNK], zd_t[:, 0:P], zd_t[:], start=True, stop=True
                )

        # ---- GroupNorm ----
        with tc.tile_pool(name="psum_tiny", bufs=1, space="PSUM") as psum_tiny:
            pe_warm(3)
            xn_tiles = []
            for t in range(CT):
                xt = x_tiles[t]
                xv = xt[:].rearrange("p (s f) -> p s f", s=2)
                st = gnp.tile([P, 2, 6], f32, tag="bnst")
                nc.vector.bn_stats(out=st[:, 0, :], in_=xv[:, 0, :])
                nc.vector.bn_stats(out=st[:, 1, :], in_=xv[:, 1, :])
                mv = gnp.tile([P, 2], f32, tag="bnmv")
                nc.vector.bn_aggr(out=mv[:], in_=st[:])
                # cst = [mean_c, var_c + mean_c^2] = [E[x], E[x^2]] per channel
                cst = gnp.tile([P, 2], f32, tag="cst")
                nc.vector.tensor_copy(cst[:, 0:1], mv[:, 0:1])
                nc.vector.scalar_tensor_tensor(
                    out=cst[:, 1:2], in0=mv[:, 0:1], scalar=mv[:, 0:1],
                    in1=mv[:, 1:2], op0=OP.mult, op1=OP.add,
                )
                # group sums (over the 16 channels of each of this tile's 8 groups)
                pgs = psum_tiny.tile([8, 2], f32, tag="pgs")
                nc.tensor.matmul(pgs[:], gmap_t[:], cst[:], start=True, stop=True)
                # mu_g, rstd_g on 8 partitions
                gr = gnp.tile([8, 2], f32, tag="gr")
                nc.vector.tensor_scalar_mul(gr[:, 0:2], pgs[:, 0:2], 1.0 / GSIZE)
                musq = gnp.tile([8, 1], f32, tag="musq")
                nc.vector.tensor_mul(musq[:], gr[:, 0:1], gr[:, 0:1])
                var = gnp.tile([8, 1], f32, tag="var")
                nc.vector.tensor_sub(var[:], gr[:, 1:2], musq[:])
                std = gnp.tile([8, 1], f32, tag="std")
                nc.scalar.activation(out=std[:], in_=var[:], func=AF.Sqrt, bias=eps_t[:])
                nc.vector.reciprocal(gr[:, 1:2], std[:])
                # broadcast mu/rstd back to the tile's 128 channels
                pbc = psum_tiny.tile([P, 2], f32, tag="pbc")
                nc.tensor.matmul(pbc[:], gmapT_t[:], gr[:], start=True, stop=True)
                scale_c = gnp.tile([P, 1], f32, tag="scale_c")
                nc.vector.tensor_mul(scale_c[:], pbc[:, 1:2], gnw_t[:, t:t + 1])
                mss = gnp.tile([P, 1], f32, tag="mss")
                nc.vector.tensor_mul(mss[:], pbc[:, 0:1], scale_c[:])
                bias_c = gnp.tile([P, 1], f32, tag="bias_c")
                nc.vector.tensor_sub(bias_c[:], gnb_t[:, t:t + 1], mss[:])
                xnt = xnp.tile([P, N], bf16, tag="xn")
                nc.vector.tensor_scalar(
                    out=xnt[:], in0=xt[:], scalar1=scale_c[:], scalar2=bias_c[:],
                    op0=OP.mult, op1=OP.add,
                )
                xn_tiles.append(xnt)
                pe_warm(2)

        with tc.tile_pool(name="psum_av", bufs=1, space="PSUM") as psum_av:

            # ---- qkv helpers ----
            def emit_vt_tile(i):
                """vT tile [128, 8*65]; for head h cols 65h..65h+64 hold v
                channels 64h..64h+64, col 65h+64 holds ones (denominator)."""
                ps = psum_big.tile([P, N], f32, tag="big", name=f"vtps{i}")
                pv = ps[:, 0:NCHUNK]
                for kk in range(CT):
                    nc.tensor.matmul(
                        pv,
                        xn_tiles[kk][:, i * P:(i + 1) * P],
                        w_tiles[kk][:, 2 * C:3 * C],
                        start=(kk == 0), stop=False,
                    )
                nc.tensor.matmul(pv, ones1_t[:], vb_t[:], start=False, stop=True)
                vt = vtp.tile([P, NHEADS, HD + 1], bf16, tag="vt", name=f"vt{i}")
                nc.vector.tensor_copy(
                    vt[:, :, 0:HD], pv.rearrange("p (h d) -> p h d", h=NHEADS)
                )
                nc.vector.tensor_copy(vt[:, :, HD:HD + 1], ones8_t[:, :, None])
                return vt

            att_tiles = []

            def emit_scores(p, i, q_t, k_t):
                """transposed scores for heads (2p, 2p+1), m-tile i -> PSUM pair."""
                pss = []
                for h in range(2):
                    ps = psum_big.tile([P, N], f32, tag="big")
                    lo = h * HD
                    for j in range(2):
                        nc.tensor.matmul(
                            ps[:, j * NCHUNK:(j + 1) * NCHUNK],
                            k_t[lo:lo + HD, i * P:(i + 1) * P],
                            q_t[lo:lo + HD, j * NCHUNK:(j + 1) * NCHUNK],
                            start=True, stop=True,
                        )
                    pss.append(ps)
                return pss

            def emit_exp(ps_pair):
                es = []
                for ps in ps_pair:
                    e = expp.tile([P, N], bf16, tag="exp")
                    nc.scalar.activation(out=e[:], in_=ps[:], func=AF.Exp, scale=1.0 / 8.0)
                    es.append(e)
                return es

            # ---- flat software-pipelined attention stream ----
            # Per A-phase step: AV_A (trailing the exp stream), one scores+exp
            # step LA ahead, and small PE filler chunks (vT production during
            # pair 0, next pair's q/k in 4-matmul chunks during steps 0..3) so
            # the PE never executes a large blocking block and the big-PSUM
            # pool keeps feeding ACT. Head B's 16 matmuls blast through the
            # retained exp tiles after head A normalizes.
            LA = 3
            steps = [(p, i) for p in range(PAIRS) for i in range(MT)]
            exps = {}
            emitted = 0

            qk_state = {}  # p -> dict(ps=[q_ps,k_ps], sb=[q_sb,k_sb], chunk=int)

            def qk_begin(p):
                qk_state[p] = {"chunk": 0, "ps": None, "sb": []}

            def qk_chunk(p):
                """Emit 2 of the 16 qk matmuls for pair p; q fully first, then
                k, so only one big-PSUM slot is held at a time."""
                st = qk_state[p]
                c = st["chunk"]
                if c >= 8:
                    return
                st["chunk"] = c + 1
                which, cc = c // 4, c % 4
                off = which * C + p * P
                if cc == 0:
                    st["ps"] = psum_big.tile(
                        [P, N], f32, tag="big", name=f"qkps{p}_{which}"
                    )
                ps = st["ps"]
                j, kks = cc // 2, (cc % 2) * 2
                for kk in (kks, kks + 1):
                    nc.tensor.matmul(
                        ps[:, j * NCHUNK:(j + 1) * NCHUNK],
                        w_tiles[kk][:, off:off + P],
                        xn_tiles[kk][:, j * NCHUNK:(j + 1) * NCHUNK],
                        start=(kk == 0), stop=(kk == CT - 1),
                    )
                if cc == 3:
                    sb = qkp.tile([P, N], bf16, tag="qk", name=f"qk{p}_{which}")
                    nc.vector.tensor_scalar_add(
                        sb[:], ps[:], qkb_t[:, which * 4 + p:which * 4 + p + 1]
                    )
                    st["sb"].append(sb)

            def qk_force(p):
                while qk_state[p]["chunk"] < 8:
                    qk_chunk(p)

            def ensure_scores(n):
                nonlocal emitted
                while emitted < min(n, len(steps)):
                    p2, i2 = steps[emitted]
                    qk_force(p2)
                    exps[(p2, i2)] = emit_exp(
                        emit_scores(p2, i2, *qk_state[p2]["sb"])
                    )
                    emitted += 1

            def emit_av(avt, p, i, h, start, stop):
                e = exps.pop((p, i))[h] if h == 1 else exps[(p, i)][h]
                for j in range(2):
                    nc.tensor.matmul(
                        avt[:, j * NCHUNK:(j + 1) * NCHUNK],
                        vt_tiles[i][:, 2 * p + h, :],
                        e[:, j * NCHUNK:(j + 1) * NCHUNK],
                        start=start, stop=stop,
                    )

            def emit_norm(att, avt, h):
                dinv = dvp.tile([1, N], f32, tag="dinv", name=f"dinv{h}")
                nc.vector.tensor_copy(dinv[:], avt[HD:HD + 1, :])
                nc.vector.reciprocal_approx_fast(dinv[:], dinv[:])
                dinvb = dvp.tile([HD, N], f32, tag="dinvb", name=f"dinvb{h}")
                nc.gpsimd.partition_broadcast(dinvb[:], dinv[:])
                nc.vector.tensor_mul(
                    att[h * HD:(h + 1) * HD, :], avt[0:HD, :], dinvb[:]
                )

            vt_tiles = [None] * MT
            proj_ps = {}
            qk_begin(0)
            qk_force(0)
            ensure_scores(LA)
            for i in range(MT):
                vt_tiles[i] = emit_vt_tile(i)
                # keep ACT's score queue topped up through the vt block
                if i in (2, 4, 6):
                    ensure_scores(LA + 1 + i // 2)
            for p in range(PAIRS):
                att = attp.tile([P, N], bf16, tag="att", name=f"att{p}")
                # head A trails the exp stream
                avt = psum_av.tile([HD + 1, N], f32, tag="av", name=f"avA{p}")
                for i in range(MT):
                    ensure_scores(p * MT + i + 1 + LA)
                    if p + 1 < PAIRS:
                        if i == 0:
                            qk_begin(p + 1)
                        qk_chunk(p + 1)
                        if i >= 2:
                            qk_chunk(p + 1)
                    emit_av(avt, p, i, 0, start=(i == 0), stop=(i == MT - 1))
                emit_norm(att, avt, 0)
                if p == PAIRS - 1:
                    # pre-accumulate proj k-steps 0..2 for o-tiles 0..2 -- keeps
                    # the PE busy while the last normalize chains run on DVE
                    for o in range(3):
                        pp = psum_big.tile([P, N], f32, tag="big", name=f"projps{o}")
                        for kk in range(CT - 1):
                            for j in range(2):
                                nc.tensor.matmul(
                                    pp[:, j * NCHUNK:(j + 1) * NCHUNK],
                                    wp_tiles[kk][:, o * P:(o + 1) * P],
                                    att_tiles[kk][:, j * NCHUNK:(j + 1) * NCHUNK],
                                    start=(kk == 0), stop=False,
                                )
                        proj_ps[o] = pp
                # head B blasts through the retained exp tiles
                avt = psum_av.tile([HD + 1, N], f32, tag="av", name=f"avB{p}")
                for i in range(MT):
                    emit_av(avt, p, i, 1, start=(i == 0), stop=(i == MT - 1))
                    if i % 3 == 2:
                        ensure_scores(p * MT + MT + i // 3 + 1 + LA)
                emit_norm(att, avt, 1)
                att_tiles.append(att)

            # ---- proj + residual ----
            for t in range(CT):
                if t in proj_ps:
                    ps = proj_ps[t]
                else:
                    ps = psum_big.tile([P, N], f32, tag="big", name=f"projfull{t}")
                    for kk in range(CT - 1):
                        for j in range(2):
                            nc.tensor.matmul(
                                ps[:, j * NCHUNK:(j + 1) * NCHUNK],
                                wp_tiles[kk][:, t * P:(t + 1) * P],
                                att_tiles[kk][:, j * NCHUNK:(j + 1) * NCHUNK],
                                start=(kk == 0), stop=False,
                            )
                for j in range(2):
                    nc.tensor.matmul(
                        ps[:, j * NCHUNK:(j + 1) * NCHUNK],
                        wp_tiles[CT - 1][:, t * P:(t + 1) * P],
                        att_tiles[CT - 1][:, j * NCHUNK:(j + 1) * NCHUNK],
                        start=False, stop=True,
                    )
                ot = outp.tile([P, N], f32, tag="ot")
                nc.vector.scalar_tensor_tensor(
                    out=ot[:], in0=ps[:], scalar=pb_t[:, t:t + 1],
                    in1=x_tiles[t][:], op0=OP.add, op1=OP.add,
                )
                nc.sync.dma_start(out_d[t * P:(t + 1) * P, :], ot[:])

    nc.compile()
    return nc


_CACHE = {}


def _get_program():
    if "nc" not in _CACHE:
        _CACHE["nc"] = build_program()
    return _CACHE["nc"]


def make_in_maps(x, gn_w, gn_b, qkv_w, qkv_b, proj_w, proj_b):
    B = x.shape[0]
    f = np.float32
    wqkvT = np.ascontiguousarray(qkv_w.T).astype(ml_dtypes.bfloat16)  # [512, 1536]
    wpT = np.ascontiguousarray(proj_w.T).astype(ml_dtypes.bfloat16)  # [512, 512]
    qkb = np.ascontiguousarray(np.asarray(qkv_b[:2 * C], f).reshape(8, P).T)  # [128, 8]
    vb = np.asarray(qkv_b[2 * C:], np.float32).reshape(1, C).astype(ml_dtypes.bfloat16)
    pb = np.ascontiguousarray(np.asarray(proj_b, f).reshape(CT, P).T)  # [128, 4]
    gnw = np.ascontiguousarray(np.asarray(gn_w, f).reshape(CT, P).T)
    gnb = np.ascontiguousarray(np.asarray(gn_b, f).reshape(CT, P).T)
    # group indicator: gmap[p, j] = 1 if channel p belongs to (tile-local) group j
    gmap = np.zeros((P, 8), f)
    gmap[np.arange(P), np.arange(P) // GSIZE] = 1.0
    gmapT = np.ascontiguousarray(gmap.T)
    ones8 = np.ones((P, 8), ml_dtypes.bfloat16)
    ones1 = np.ones((1, P), ml_dtypes.bfloat16)
    shared = dict(
        wqkvT=wqkvT, wpT=wpT, qkb=qkb, vb=vb, pb=pb, gnw=gnw, gnb=gnb,
        gmap=gmap, gmapT=gmapT, ones8=ones8, ones1=ones1,
    )
    xs = np.asarray(x, f).reshape(B, C, N)
    return [dict(shared, x=np.ascontiguousarray(xs[i])) for i in range(B)]


def run(in_maps, trace=False, **kw):
    nc = _get_program()
    return run_bass_kernel_spmd(nc, in_maps, core_ids=list(range(len(in_maps))), trace=trace, **kw)


def kernel(x, gn_w, gn_b, qkv_w, qkv_b, proj_w, proj_b):
    x = np.asarray(x)
    B, c, h, w = x.shape
    in_maps = make_in_maps(x, gn_w, gn_b, qkv_w, qkv_b, proj_w, proj_b)
    res = run(in_maps)
    out = np.stack([res.results[i]["out"].reshape(c, h, w) for i in range(B)])
    return out.astype(np.float32)
